# revision 19
# baseline (speedup 1.0000x reference)
"""DistMult edge-scoring kernel for Trainium2 (8 NeuronCores, SPMD).

score[j] = sum_d emb_A[a_idx[j], d] * k[d] * emb_B[b_idx[j], d]
for 9E pairs: E positive edges, 4E head-corrupted, 4E tail-corrupted.

Strategy (v4, fully host-gathered dense bf16 — exploits per-edge row
reuse):
- Per edge e the 9 scores are dots against just 10 rows:
  ak = k*A[ep0[e]], b = B[ep1[e]], hk_j = k*A[head[e,j]],
  t_j = B[tail[e,j]];  pos = ak.b, head_j = hk_j.b, tail_j = ak.t_j.
  The baseline streamed 18 rows/edge; this streams 10.
- ALL rows are gathered on the host (numpy fancy indexing) into one
  dense bf16 array per core, so the device does zero SWDGE gathers
  (the baseline's Q7-descriptor bottleneck) and half the bytes
  (bf16 vs f32; rel_norm error ~2.3e-3, well under the 2e-2 gate).
- Edges are dealt round-robin across the 8 cores in 128-edge blocks;
  every core runs an identical program on 98 slots (12544 edges).
- Per slot: one [128, 1280] bf16 tile (10 rows x 128 d per edge
  partition); 9 fused scalar_tensor_tensor (mul + accumulate-reduce)
  ops produce the 9 score columns. Double-buffered 14-slot DMA
  batches (5.25 MB each) overlap load with compute.
- Host inverse-deals the [128, S*9] score tiles back to reference
  order.
"""

import numpy as np

# problem constants
N_A = 100000
N_B = 100000
D = 128
E = 100000
NEG = 4
NCORES = 8

P = 128
S = 98                 # slots (128-edge blocks) per core: 98*8*128 >= E
BS = 14                # slots per DMA batch (98 = 7*14)
ROW = 10 * D           # free-dim elements per edge: ak|hk0..3|b|t0..3

_CACHED = {}


def _build_program():
    import concourse.tile as tile
    from concourse import bacc, mybir

    f32 = mybir.dt.float32  # noqa: F841
    bf = mybir.dt.bfloat16
    mult = mybir.AluOpType.mult
    add = mybir.AluOpType.add

    nc = bacc.Bacc("TRN2", target_bir_lowering=False, debug=False,
                   num_devices=NCORES)
    x_d = nc.dram_tensor("x", [P, S * ROW], bf, kind="ExternalInput").ap()
    s_out = nc.dram_tensor("scores", [P, S * 9], bf,
                           kind="ExternalOutput").ap()

    with tile.TileContext(nc) as tc:
        with (
            tc.tile_pool(name="in", bufs=3) as in_pool,
            tc.tile_pool(name="trash", bufs=1) as trash_pool,
            tc.tile_pool(name="sc", bufs=1) as s_pool,
        ):
            # scores col layout: per batch bi a block of n*9 columns:
            # first n*5 = (slot-local s, [pos, head0..3]),
            # then  n*4 = (slot-local s, [tail0..3]).
            # bf16 everywhere keeps every DVE operand 2-byte so the
            # multiply AND the whole add-tree run in packed 2x mode
            # (TENSOR_REDUCE has no 2x uop -> replaced by a TT-add tree).
            scores = s_pool.tile([P, S * 9], bf)
            nb = (S + BS - 1) // BS
            for bi in range(nb):
                n = min(BS, S - bi * BS)
                h = n // 2
                C = n * 9
                T = in_pool.tile([P, BS * ROW], bf, tag="x")
                # split each batch across the two HWDGE rings
                # (sync + scalar) so both generate descriptors
                nc.sync.dma_start(
                    T[:, 0:h * ROW],
                    x_d[:, bi * BS * ROW:(bi * BS + h) * ROW])
                nc.scalar.dma_start(
                    T[:, h * ROW:n * ROW],
                    x_d[:, (bi * BS + h) * ROW:(bi * BS + n) * ROW])
                R = T[:, 0:n * ROW].rearrange("p (s t d) -> p s t d",
                                              t=10, d=D)
                lhs1 = R[:, :, 0:5, :]                       # ak|hk0..3
                b_b = R[:, :, 5:6, :].broadcast_to([P, n, 5, D])
                lhs2 = R[:, :, 6:10, :]                      # t0..3
                ak_b = R[:, :, 0:1, :].broadcast_to([P, n, 4, D])
                pr = trash_pool.tile([P, BS * 9 * D], bf, tag="pr")
                p1 = pr[:, 0:n * 5 * D].rearrange(
                    "p (s t d) -> p s t d", t=5, d=D)
                p2 = pr[:, n * 5 * D:n * 9 * D].rearrange(
                    "p (s t d) -> p s t d", t=4, d=D)
                nc.vector.tensor_tensor(out=p1, in0=lhs1, in1=b_b, op=mult)
                nc.vector.tensor_tensor(out=p2, in0=lhs2, in1=ak_b, op=mult)
                # binary add-tree over d (2x packed TT adds), stopped at
                # width 4 to keep every operand 4B-aligned with innermost
                # count >= 2; a final cheap 1x reduce finishes 4 -> 1.
                cur = pr[:, 0:C * D].rearrange("p (c d) -> p c d", d=D)
                w = D
                while w > 4:
                    hw_ = w // 2
                    nxt_t = trash_pool.tile([P, BS * 9 * hw_], bf,
                                            tag=f"h{hw_}")
                    nxt = nxt_t[:, 0:C * hw_].rearrange(
                        "p (c d) -> p c d", d=hw_)
                    # offload the 32- and 16-wide levels to the otherwise
                    # idle GPSIMD engine (DVE is the bottleneck)
                    eng = nc.gpsimd if hw_ in (32, 16) else nc.vector
                    eng.tensor_tensor(
                        out=nxt, in0=cur[:, :, 0:hw_],
                        in1=cur[:, :, hw_:w], op=add)
                    cur = nxt
                    w = hw_
                with nc.allow_low_precision(
                        reason="bf16 scores; reduce accumulates fp32 "
                               "internally, only the final write rounds"):
                    nc.vector.reduce_sum(
                        out=scores[:, bi * BS * 9:bi * BS * 9 + C],
                        in_=cur, axis=mybir.AxisListType.X)

            nc.sync.dma_start(s_out[:], scores[:])

    nc.compile()
    return nc


def kernel(emb_A, emb_B, rel_kernel, edge_pos, head_batch, tail_batch):
    import ml_dtypes
    from concourse.bass_utils import run_bass_kernel_spmd

    bf16 = ml_dtypes.bfloat16
    emb_A = np.asarray(emb_A, dtype=np.float32)
    emb_B = np.asarray(emb_B, dtype=np.float32)
    kv = np.asarray(rel_kernel, dtype=np.float32)[0]
    ep = np.asarray(edge_pos, dtype=np.int64)
    hb = np.asarray(head_batch, dtype=np.int64)
    tb = np.asarray(tail_batch, dtype=np.int64)

    # prescale k into the A-side table once; round both tables to bf16
    eAk = (emb_A * kv[None, :]).astype(bf16)   # [N_A, D]
    eB16 = emb_B.astype(bf16)                  # [N_B, D]

    # edge e lives at (core c, slot s, partition p): e = (s*8+c)*128 + p
    p_arr = np.arange(P)
    s_arr = np.arange(S)
    in_maps = []
    for c in range(NCORES):
        e = ((s_arr * NCORES + c)[:, None] * P + p_arr[None, :]).ravel()
        esafe = np.where(e < E, e, 0)
        ia = np.empty((S * P, 5), np.int64)
        ia[:, 0] = ep[0][esafe]
        ia[:, 1:] = hb[esafe]
        ib = np.empty((S * P, 5), np.int64)
        ib[:, 0] = ep[1][esafe]
        ib[:, 1:] = tb[esafe]
        x = np.empty((S * P, 10, D), bf16)
        x[:, 0] = eAk[ia[:, 0]]
        x[:, 1:5] = eAk[ia[:, 1:]]
        x[:, 5] = eB16[ib[:, 0]]
        x[:, 6:] = eB16[ib[:, 1:]]
        # [s*P+p, row] -> [p, s*ROW] partition-major device layout
        x = np.ascontiguousarray(
            x.reshape(S, P, ROW).transpose(1, 0, 2).reshape(P, S * ROW))
        in_maps.append({"x": x})

    sig = ("v8", S, BS)
    if _CACHED.get("sig") != sig:
        _CACHED["nc"] = _build_program()
        _CACHED["sig"] = sig
    nc = _CACHED["nc"]
    _CACHED["in_maps"] = in_maps
    _CACHED["plan"] = sig

    res = run_bass_kernel_spmd(nc, in_maps, core_ids=list(range(NCORES)))
    _CACHED["last_results"] = res

    out = np.empty(9 * E, dtype=np.float32)
    for c in range(NCORES):
        flat = res.results[c]["scores"].astype(np.float32)
        blk = flat.reshape(P, S // BS, BS * 9)
        sc1 = blk[:, :, :BS * 5].reshape(P, S, 5)   # pos|head0..3
        sc2 = blk[:, :, BS * 5:].reshape(P, S, 4)   # tail0..3
        e = (s_arr * NCORES + c)[None, :] * P + p_arr[:, None]  # [p, s]
        valid = e < E
        ev = e[valid]
        out[ev] = sc1[:, :, 0][valid]
        for j in range(4):
            out[E + ev * 4 + j] = sc1[:, :, 1 + j][valid]
            out[5 * E + ev * 4 + j] = sc2[:, :, j][valid]
    return out


# revision 21
# speedup vs baseline: 1.3222x; 1.3222x over previous
"""DistMult edge-scoring kernel for Trainium2 (8 NeuronCores, SPMD).

score[j] = sum_d emb_A[a_idx[j], d] * k[d] * emb_B[b_idx[j], d]
for 9E pairs: E positive edges, 4E head-corrupted, 4E tail-corrupted.

Strategy (v4, fully host-gathered dense bf16 — exploits per-edge row
reuse):
- Per edge e the 9 scores are dots against just 10 rows:
  ak = k*A[ep0[e]], b = B[ep1[e]], hk_j = k*A[head[e,j]],
  t_j = B[tail[e,j]];  pos = ak.b, head_j = hk_j.b, tail_j = ak.t_j.
  The baseline streamed 18 rows/edge; this streams 10.
- ALL rows are gathered on the host (numpy fancy indexing) into one
  dense bf16 array per core, so the device does zero SWDGE gathers
  (the baseline's Q7-descriptor bottleneck) and half the bytes
  (bf16 vs f32; rel_norm error ~2.3e-3, well under the 2e-2 gate).
- Edges are dealt round-robin across the 8 cores in 128-edge blocks;
  every core runs an identical program on 98 slots (12544 edges).
- Per slot: one [128, 1280] bf16 tile (10 rows x 128 d per edge
  partition); 9 fused scalar_tensor_tensor (mul + accumulate-reduce)
  ops produce the 9 score columns. Double-buffered 14-slot DMA
  batches (5.25 MB each) overlap load with compute.
- Host inverse-deals the [128, S*9] score tiles back to reference
  order.
"""

import numpy as np

# problem constants
N_A = 100000
N_B = 100000
D = 128
E = 100000
NEG = 4
NCORES = 8

P = 128
S = 98                 # slots (128-edge blocks) per core: 98*8*128 >= E
BS = 14                # slots per DMA batch (98 = 7*14)
ROW = 10 * D           # free-dim elements per edge: ak|hk0..3|b|t0..3

_CACHED = {}


def _build_program():
    import concourse.tile as tile
    from concourse import bacc, mybir

    f32 = mybir.dt.float32  # noqa: F841
    bf = mybir.dt.bfloat16
    mult = mybir.AluOpType.mult
    add = mybir.AluOpType.add

    nc = bacc.Bacc("TRN2", target_bir_lowering=False, debug=False,
                   num_devices=NCORES)
    x_d = nc.dram_tensor("x", [P, S * ROW], bf, kind="ExternalInput").ap()
    s_out = nc.dram_tensor("scores", [P, S * 9], bf,
                           kind="ExternalOutput").ap()

    with tile.TileContext(nc) as tc:
        with (
            tc.tile_pool(name="in", bufs=3) as in_pool,
            tc.tile_pool(name="trash", bufs=1) as trash_pool,
            tc.tile_pool(name="sc", bufs=1) as s_pool,
        ):
            # scores col layout: per batch bi a block of n*9 columns:
            # first n*5 = (slot-local s, [pos, head0..3]),
            # then  n*4 = (slot-local s, [tail0..3]).
            # bf16 everywhere keeps every DVE operand 2-byte so the
            # multiply AND the whole add-tree run in packed 2x mode
            # (TENSOR_REDUCE has no 2x uop -> replaced by a TT-add tree).
            scores = s_pool.tile([P, S * 9], bf)
            nb = (S + BS - 1) // BS
            for bi in range(nb):
                n = min(BS, S - bi * BS)
                h = n // 2
                C = n * 9
                T = in_pool.tile([P, BS * ROW], bf, tag="x")
                # split each batch across the two HWDGE rings
                # (sync + scalar) so both generate descriptors
                nc.sync.dma_start(
                    T[:, 0:h * ROW],
                    x_d[:, bi * BS * ROW:(bi * BS + h) * ROW])
                nc.scalar.dma_start(
                    T[:, h * ROW:n * ROW],
                    x_d[:, (bi * BS + h) * ROW:(bi * BS + n) * ROW])
                R = T[:, 0:n * ROW].rearrange("p (s t d) -> p s t d",
                                              t=10, d=D)
                lhs1 = R[:, :, 0:5, :]                       # ak|hk0..3
                b_b = R[:, :, 5:6, :].broadcast_to([P, n, 5, D])
                lhs2 = R[:, :, 6:10, :]                      # t0..3
                ak_b = R[:, :, 0:1, :].broadcast_to([P, n, 4, D])
                pr = trash_pool.tile([P, BS * 9 * D], bf, tag="pr")
                p1 = pr[:, 0:n * 5 * D].rearrange(
                    "p (s t d) -> p s t d", t=5, d=D)
                p2 = pr[:, n * 5 * D:n * 9 * D].rearrange(
                    "p (s t d) -> p s t d", t=4, d=D)
                nc.vector.tensor_tensor(out=p1, in0=lhs1, in1=b_b, op=mult)
                nc.vector.tensor_tensor(out=p2, in0=lhs2, in1=ak_b, op=mult)
                # binary add-tree over d (2x packed TT adds), stopped at
                # width 4 to keep every operand 4B-aligned with innermost
                # count >= 2; a final cheap 1x reduce finishes 4 -> 1.
                cur = pr[:, 0:C * D].rearrange("p (c d) -> p c d", d=D)
                w = D
                while w > 4:
                    hw_ = w // 2
                    nxt_t = trash_pool.tile([P, BS * 9 * hw_], bf,
                                            tag=f"h{hw_}")
                    nxt = nxt_t[:, 0:C * hw_].rearrange(
                        "p (c d) -> p c d", d=hw_)
                    # (GPSIMD offload measured slower: Pool TT ~7.8us/op
                    # AND it contends for the shared DVE SBUF port)
                    nc.vector.tensor_tensor(
                        out=nxt, in0=cur[:, :, 0:hw_],
                        in1=cur[:, :, hw_:w], op=add)
                    cur = nxt
                    w = hw_
                with nc.allow_low_precision(
                        reason="bf16 scores; reduce accumulates fp32 "
                               "internally, only the final write rounds"):
                    nc.vector.reduce_sum(
                        out=scores[:, bi * BS * 9:bi * BS * 9 + C],
                        in_=cur, axis=mybir.AxisListType.X)

            nc.sync.dma_start(s_out[:], scores[:])

    nc.compile()
    return nc


def kernel(emb_A, emb_B, rel_kernel, edge_pos, head_batch, tail_batch):
    import ml_dtypes
    from concourse.bass_utils import run_bass_kernel_spmd

    bf16 = ml_dtypes.bfloat16
    emb_A = np.asarray(emb_A, dtype=np.float32)
    emb_B = np.asarray(emb_B, dtype=np.float32)
    kv = np.asarray(rel_kernel, dtype=np.float32)[0]
    ep = np.asarray(edge_pos, dtype=np.int64)
    hb = np.asarray(head_batch, dtype=np.int64)
    tb = np.asarray(tail_batch, dtype=np.int64)

    # prescale k into the A-side table once; round both tables to bf16
    eAk = (emb_A * kv[None, :]).astype(bf16)   # [N_A, D]
    eB16 = emb_B.astype(bf16)                  # [N_B, D]

    # edge e lives at (core c, slot s, partition p): e = (s*8+c)*128 + p
    p_arr = np.arange(P)
    s_arr = np.arange(S)
    in_maps = []
    for c in range(NCORES):
        e = ((s_arr * NCORES + c)[:, None] * P + p_arr[None, :]).ravel()
        esafe = np.where(e < E, e, 0)
        ia = np.empty((S * P, 5), np.int64)
        ia[:, 0] = ep[0][esafe]
        ia[:, 1:] = hb[esafe]
        ib = np.empty((S * P, 5), np.int64)
        ib[:, 0] = ep[1][esafe]
        ib[:, 1:] = tb[esafe]
        x = np.empty((S * P, 10, D), bf16)
        x[:, 0] = eAk[ia[:, 0]]
        x[:, 1:5] = eAk[ia[:, 1:]]
        x[:, 5] = eB16[ib[:, 0]]
        x[:, 6:] = eB16[ib[:, 1:]]
        # [s*P+p, row] -> [p, s*ROW] partition-major device layout
        x = np.ascontiguousarray(
            x.reshape(S, P, ROW).transpose(1, 0, 2).reshape(P, S * ROW))
        in_maps.append({"x": x})

    sig = ("v9", S, BS)
    if _CACHED.get("sig") != sig:
        _CACHED["nc"] = _build_program()
        _CACHED["sig"] = sig
    nc = _CACHED["nc"]
    _CACHED["in_maps"] = in_maps
    _CACHED["plan"] = sig

    res = run_bass_kernel_spmd(nc, in_maps, core_ids=list(range(NCORES)))
    _CACHED["last_results"] = res

    out = np.empty(9 * E, dtype=np.float32)
    for c in range(NCORES):
        flat = res.results[c]["scores"].astype(np.float32)
        blk = flat.reshape(P, S // BS, BS * 9)
        sc1 = blk[:, :, :BS * 5].reshape(P, S, 5)   # pos|head0..3
        sc2 = blk[:, :, BS * 5:].reshape(P, S, 4)   # tail0..3
        e = (s_arr * NCORES + c)[None, :] * P + p_arr[:, None]  # [p, s]
        valid = e < E
        ev = e[valid]
        out[ev] = sc1[:, :, 0][valid]
        for j in range(4):
            out[E + ev * 4 + j] = sc1[:, :, 1 + j][valid]
            out[5 * E + ev * 4 + j] = sc2[:, :, j][valid]
    return out


# revision 26
# speedup vs baseline: 1.4679x; 1.1102x over previous
"""DistMult edge-scoring kernel for Trainium2 (8 NeuronCores, SPMD).

score[j] = sum_d emb_A[a_idx[j], d] * k[d] * emb_B[b_idx[j], d]
for 9E pairs: E positive edges, 4E head-corrupted, 4E tail-corrupted.

Strategy (v4, fully host-gathered dense bf16 — exploits per-edge row
reuse):
- Per edge e the 9 scores are dots against just 10 rows:
  ak = k*A[ep0[e]], b = B[ep1[e]], hk_j = k*A[head[e,j]],
  t_j = B[tail[e,j]];  pos = ak.b, head_j = hk_j.b, tail_j = ak.t_j.
  The baseline streamed 18 rows/edge; this streams 10.
- ALL rows are gathered on the host (numpy fancy indexing) into one
  dense bf16 array per core, so the device does zero SWDGE gathers
  (the baseline's Q7-descriptor bottleneck) and half the bytes
  (bf16 vs f32; rel_norm error ~2.3e-3, well under the 2e-2 gate).
- Edges are dealt round-robin across the 8 cores in 128-edge blocks;
  every core runs an identical program on 98 slots (12544 edges).
- Per slot: one [128, 1280] bf16 tile (10 rows x 128 d per edge
  partition); 9 fused scalar_tensor_tensor (mul + accumulate-reduce)
  ops produce the 9 score columns. Double-buffered 14-slot DMA
  batches (5.25 MB each) overlap load with compute.
- Host inverse-deals the [128, S*9] score tiles back to reference
  order.
"""

import numpy as np

# problem constants
N_A = 100000
N_B = 100000
D = 128
E = 100000
NEG = 4
NCORES = 8

P = 128
S = 98                 # slots (128-edge blocks) per core: 98*8*128 >= E
BS = 14                # max slots per DMA batch
# geometric ramp: small first batches so compute starts ~2us after the
# first DMA instead of waiting for a full 5.25MB batch; DVE is ~1.22x
# slower per slot than DMA, so sizes can grow ~22% per batch.
BATCHES = [2, 4, 6, 8, 10, 12, 14, 14, 14, 14]
assert sum(BATCHES) == S
ROW = 10 * D           # free-dim elements per edge: ak|hk0..3|b|t0..3

_CACHED = {}


def _build_program():
    import concourse.tile as tile
    from concourse import bacc, mybir

    f32 = mybir.dt.float32  # noqa: F841
    bf = mybir.dt.bfloat16
    mult = mybir.AluOpType.mult
    add = mybir.AluOpType.add

    nc = bacc.Bacc("TRN2", target_bir_lowering=False, debug=False,
                   num_devices=NCORES)
    x_d = nc.dram_tensor("x", [P, S * ROW], bf, kind="ExternalInput").ap()
    s_out = nc.dram_tensor("scores", [P, S * 9], bf,
                           kind="ExternalOutput").ap()

    with tile.TileContext(nc) as tc:
        with (
            tc.tile_pool(name="in", bufs=2) as in_pool,
            tc.tile_pool(name="trash", bufs=1) as trash_pool,
            tc.tile_pool(name="sc", bufs=1) as s_pool,
        ):
            # scores col layout: per batch bi a block of n*9 columns:
            # first n*5 = (slot-local s, [pos, head0..3]),
            # then  n*4 = (slot-local s, [tail0..3]).
            # bf16 everywhere keeps every DVE operand 2-byte so the
            # multiply AND the whole add-tree run in packed 2x mode
            # (TENSOR_REDUCE has no 2x uop -> replaced by a TT-add tree).
            scores = s_pool.tile([P, S * 9], bf)
            s0 = 0
            for bi, n in enumerate(BATCHES):
                h = n // 2
                C = n * 9
                T = in_pool.tile([P, BS * ROW], bf, tag="x")
                # split each batch across the two HWDGE rings
                # (sync + scalar) so both generate descriptors
                nc.sync.dma_start(
                    T[:, 0:h * ROW],
                    x_d[:, s0 * ROW:(s0 + h) * ROW])
                nc.scalar.dma_start(
                    T[:, h * ROW:n * ROW],
                    x_d[:, (s0 + h) * ROW:(s0 + n) * ROW])
                R = T[:, 0:n * ROW].rearrange("p (s t d) -> p s t d",
                                              t=10, d=D)
                lhs1 = R[:, :, 0:5, :]                       # ak|hk0..3
                b_b = R[:, :, 5:6, :].broadcast_to([P, n, 5, D])
                lhs2 = R[:, :, 6:10, :]                      # t0..3
                ak_b = R[:, :, 0:1, :].broadcast_to([P, n, 4, D])
                pr = trash_pool.tile([P, BS * 9 * D], bf, tag="pr")
                p1 = pr[:, 0:n * 5 * D].rearrange(
                    "p (s t d) -> p s t d", t=5, d=D)
                p2 = pr[:, n * 5 * D:n * 9 * D].rearrange(
                    "p (s t d) -> p s t d", t=4, d=D)
                nc.vector.tensor_tensor(out=p1, in0=lhs1, in1=b_b, op=mult)
                nc.vector.tensor_tensor(out=p2, in0=lhs2, in1=ak_b, op=mult)
                # binary add-tree over d (2x packed TT adds), stopped at
                # width 4 to keep every operand 4B-aligned with innermost
                # count >= 2; a final cheap 1x reduce finishes 4 -> 1.
                cur = pr[:, 0:C * D].rearrange("p (c d) -> p c d", d=D)
                w = D
                while w > 4:
                    hw_ = w // 2
                    nxt_t = trash_pool.tile([P, BS * 9 * hw_], bf,
                                            tag=f"h{hw_}")
                    nxt = nxt_t[:, 0:C * hw_].rearrange(
                        "p (c d) -> p c d", d=hw_)
                    # (GPSIMD offload measured slower: Pool TT ~7.8us/op
                    # AND it contends for the shared DVE SBUF port)
                    nc.vector.tensor_tensor(
                        out=nxt, in0=cur[:, :, 0:hw_],
                        in1=cur[:, :, hw_:w], op=add)
                    cur = nxt
                    w = hw_
                with nc.allow_low_precision(
                        reason="bf16 scores; reduce accumulates fp32 "
                               "internally, only the final write rounds"):
                    nc.vector.reduce_sum(
                        out=scores[:, s0 * 9:s0 * 9 + C],
                        in_=cur, axis=mybir.AxisListType.X)
                # stream this batch's scores out now so the final DMA
                # only covers the last batch (shorter tail)
                nc.sync.dma_start(s_out[:, s0 * 9:s0 * 9 + C],
                                  scores[:, s0 * 9:s0 * 9 + C])
                s0 += n

    nc.compile()
    return nc


def kernel(emb_A, emb_B, rel_kernel, edge_pos, head_batch, tail_batch):
    import ml_dtypes
    from concourse.bass_utils import run_bass_kernel_spmd

    bf16 = ml_dtypes.bfloat16
    emb_A = np.asarray(emb_A, dtype=np.float32)
    emb_B = np.asarray(emb_B, dtype=np.float32)
    kv = np.asarray(rel_kernel, dtype=np.float32)[0]
    ep = np.asarray(edge_pos, dtype=np.int64)
    hb = np.asarray(head_batch, dtype=np.int64)
    tb = np.asarray(tail_batch, dtype=np.int64)

    # prescale k into the A-side table once; round both tables to bf16
    eAk = (emb_A * kv[None, :]).astype(bf16)   # [N_A, D]
    eB16 = emb_B.astype(bf16)                  # [N_B, D]

    # edge e lives at (core c, slot s, partition p): e = (s*8+c)*128 + p
    p_arr = np.arange(P)
    s_arr = np.arange(S)
    in_maps = []
    for c in range(NCORES):
        e = ((s_arr * NCORES + c)[:, None] * P + p_arr[None, :]).ravel()
        esafe = np.where(e < E, e, 0)
        ia = np.empty((S * P, 5), np.int64)
        ia[:, 0] = ep[0][esafe]
        ia[:, 1:] = hb[esafe]
        ib = np.empty((S * P, 5), np.int64)
        ib[:, 0] = ep[1][esafe]
        ib[:, 1:] = tb[esafe]
        x = np.empty((S * P, 10, D), bf16)
        x[:, 0] = eAk[ia[:, 0]]
        x[:, 1:5] = eAk[ia[:, 1:]]
        x[:, 5] = eB16[ib[:, 0]]
        x[:, 6:] = eB16[ib[:, 1:]]
        # [s*P+p, row] -> [p, s*ROW] partition-major device layout
        x = np.ascontiguousarray(
            x.reshape(S, P, ROW).transpose(1, 0, 2).reshape(P, S * ROW))
        in_maps.append({"x": x})

    sig = ("v10", S, tuple(BATCHES))
    if _CACHED.get("sig") != sig:
        _CACHED["nc"] = _build_program()
        _CACHED["sig"] = sig
    nc = _CACHED["nc"]
    _CACHED["in_maps"] = in_maps
    _CACHED["plan"] = sig

    res = run_bass_kernel_spmd(nc, in_maps, core_ids=list(range(NCORES)))
    _CACHED["last_results"] = res

    out = np.empty(9 * E, dtype=np.float32)
    for c in range(NCORES):
        flat = res.results[c]["scores"].astype(np.float32)
        sc1_parts, sc2_parts = [], []
        o = 0
        for n in BATCHES:
            blk = flat[:, o * 9:(o + n) * 9]
            sc1_parts.append(blk[:, :n * 5].reshape(P, n, 5))
            sc2_parts.append(blk[:, n * 5:].reshape(P, n, 4))
            o += n
        sc1 = np.concatenate(sc1_parts, axis=1)   # [P, S, 5] pos|head0..3
        sc2 = np.concatenate(sc2_parts, axis=1)   # [P, S, 4] tail0..3
        e = (s_arr * NCORES + c)[None, :] * P + p_arr[:, None]  # [p, s]
        valid = e < E
        ev = e[valid]
        out[ev] = sc1[:, :, 0][valid]
        for j in range(4):
            out[E + ev * 4 + j] = sc1[:, :, 1 + j][valid]
            out[5 * E + ev * 4 + j] = sc2[:, :, j][valid]
    return out
